# revision 4
# baseline (speedup 1.0000x reference)
"""Bidirectional 2-layer LSTM (H=1024, SEQ=1024, batch 1) on 8 trn2 NeuronCores.

Strategy: the recurrence is serial, so each (direction, layer) chain runs
whole on one core with bf16 weights resident in SBUF (fused LDW+matmul pairs
for the Whh matvec). Layers are software-pipelined across cores: the layer-0
core streams its h history to the layer-1 core in 16-step windows via
remote SBUF-to-SBUF DMA (latency-tolerant), and each core computes its own
input projection Wx just-in-time with interleaved batched matmuls.
  core 0: fw layer0   core 1: fw layer1
  core 2: bw layer0   core 3: bw layer1   (4-7: mirrors, ignored)
State c is fp32; matmul operands bf16; gates accumulate in fp32 PSUM.
"""
import sys
import numpy as np

sys.path.insert(0, "/opt/trn_rl_repo")

import ml_dtypes
import concourse.bass as bass
import concourse.bacc as bacc
import concourse.mybir as mybir
from concourse.bass_utils import run_bass_kernel_spmd

F32 = mybir.dt.float32
BF16 = mybir.dt.bfloat16
I32 = mybir.dt.int32

H = 1024
GD = 4 * H
KCH = 8
MB = 32
G = 16
NSTEPS = 1024


def _build(nsteps=NSTEPS, tail_delay_cycles=0):
    NW = nsteps // G
    nc = bacc.Bacc("TRN2", target_bir_lowering=False, debug=False, num_devices=8)

    whhT_d = nc.dram_tensor("whhT", [128, KCH * GD], BF16, kind="ExternalInput")
    wihT_d = nc.dram_tensor("wihT", [128, KCH * GD], BF16, kind="ExternalInput")
    strm_d = nc.dram_tensor("strm", [128, nsteps * 8], BF16, kind="ExternalInput")
    bias_d = nc.dram_tensor("bias", [128, MB], F32, kind="ExternalInput")
    role_d = nc.dram_tensor("role", [1, 2], I32, kind="ExternalInput")
    hist_o = nc.dram_tensor("hist_out", [128, (nsteps + 1) * 8], BF16, kind="ExternalOutput")
    c_o = nc.dram_tensor("c_out", [128, 8], F32, kind="ExternalOutput")

    whhT = nc.alloc_sbuf_tensor("whhT_sb", [128, KCH * GD], BF16).ap()
    wihT = nc.alloc_sbuf_tensor("wihT_sb", [128, KCH * GD], BF16).ap()
    strm = nc.alloc_sbuf_tensor("strm_sb", [128, nsteps, 8], BF16).ap()
    hist = nc.alloc_sbuf_tensor("hist_sb", [128, nsteps + 1, 8], BF16).ap()
    wx = nc.alloc_sbuf_tensor("wx_sb", [128, MB, 4, G], F32).ap()
    bias = nc.alloc_sbuf_tensor("bias_sb", [128, MB], F32).ap()
    role = nc.alloc_sbuf_tensor("role_sb", [1, 2], I32).ap()
    g_sb = nc.alloc_sbuf_tensor("g_sb", [128, MB], F32).ap()
    g2_sb = nc.alloc_sbuf_tensor("g2_sb", [128, 8], F32).ap()
    s_sb = nc.alloc_sbuf_tensor("s_sb", [128, 24], F32).ap()
    tg_sb = nc.alloc_sbuf_tensor("tg_sb", [128, 8], F32).ap()
    t1_sb = nc.alloc_sbuf_tensor("t1_sb", [128, 8], F32).ap()
    t2_sb = nc.alloc_sbuf_tensor("t2_sb", [128, 8], F32).ap()
    c_sb = nc.alloc_sbuf_tensor("c_sb", [128, 8], F32).ap()
    tc_sb = nc.alloc_sbuf_tensor("tc_sb", [128, 8], F32).ap()
    dummy = nc.alloc_sbuf_tensor("dummy_sb", [128, 8], F32).ap()

    pch = [nc.alloc_psum_tensor(f"pch{i}", [128, 512], F32).ap() for i in range(2)]
    pgg = [nc.alloc_psum_tensor(f"pgg{i}", [128, 512], F32).ap() for i in range(2)]
    pax = [nc.alloc_psum_tensor(f"pax{i}", [128, 512], F32).ap() for i in range(2)]

    def wblk(w, kk, m):
        off = kk * GD + m * 128
        return w[:, off:off + 128]

    with (
        nc.Block() as block,
        nc.semaphore("dma_in") as dma_in,
        nc.semaphore("init_ok") as init_ok,
        nc.semaphore("gates_done") as gates_done,
        nc.semaphore("g_ready") as g_ready,
        nc.semaphore("gpsum_g") as gpsum_g,
        nc.semaphore("g2_ready") as g2_ready,
        nc.semaphore("s_ready") as s_ready,
        nc.semaphore("tg_ready") as tg_ready,
        nc.semaphore("dve1") as dve1,
        nc.semaphore("c_done") as c_done,
        nc.semaphore("tc_done") as tc_done,
        nc.semaphore("h_done") as h_done,
        nc.semaphore("aux_done") as aux_done,
        nc.semaphore("aux_freed") as aux_freed,
        nc.semaphore("stream_ok") as stream_ok,
        nc.semaphore("rsem") as rsem,
        nc.semaphore("peer_rdy") as peer_rdy,
        nc.semaphore("psem") as psem,
        nc.semaphore("lsem") as lsem,
        nc.semaphore("out_dma") as out_dma,
    ):
        @block.sync
        def _(s):
            for half in range(2):
                off = half * (KCH * GD // 2)
                sz = KCH * GD // 2
                s.dma_start(out=whhT[:, off:off + sz], in_=whhT_d.ap()[:, off:off + sz]).then_inc(dma_in, 16)
                s.dma_start(out=wihT[:, off:off + sz], in_=wihT_d.ap()[:, off:off + sz]).then_inc(dma_in, 16)
            s.dma_start(out=strm[:, :, :], in_=strm_d.ap()).then_inc(dma_in, 16)
            s.dma_start(out=bias[:], in_=bias_d.ap()).then_inc(dma_in, 16)
            s.dma_start(out=role[:], in_=role_d.ap()).then_inc(dma_in, 16)
            s.wait_ge(h_done, nsteps)
            s.wait_ge(c_done, nsteps)
            s.dma_start(out=hist_o.ap(), in_=hist[:, :, :]).then_inc(out_dma, 16)
            s.dma_start(out=c_o.ap(), in_=c_sb[:]).then_inc(out_dma, 16)
            s.wait_ge(out_dma, 32)

        NDMA_IN = 7

        @block.gpsimd
        def _(g):
            g.memset(hist[:, 0, :], 0).then_inc(init_ok, 1)
            g.memset(c_sb[:], 0).then_inc(init_ok, 1)
            g.memset(dummy[:], 0).then_inc(init_ok, 1)
            g.wait_ge(dma_in, NDMA_IN * 16)
            # handshake: don't let data sends land before the peer's stream
            # preload DMA has finished
            g.remote_sem_update_broadcast(
                remote_sem=peer_rdy, local_sem=lsem,
                rdests=[None, (0, 1)] + [None] * 6,
            ).then_inc(psem, 1)
            g.wait_ge(psem, 1)
            g.trigger_dma(count=1)
            g.wait_ge(peer_rdy, 2)
            with g.register("arr_th") as arr_th, g.register("arr_step") as arr_step:
                g.reg_load(arr_step, role[0:1, 0:1])
                g.reg_load(arr_th, role[0:1, 0:1])
                for w in range(min(2, NW)):
                    g.wait_ge(rsem, arr_th)
                    g.reg_add(arr_th, arr_th, arr_step)
                    g.sem_inc(stream_ok, 1)
                for w in range(NW):
                    if w + 2 < NW:
                        g.wait_ge(rsem, arr_th)
                        g.reg_add(arr_th, arr_th, arr_step)
                        g.sem_inc(stream_ok, 1)
                    # aux_done wait orders peer garbage-writes after our own
                    # aux reads of the same stream window
                    g.wait_ge(aux_done, 32 * (w + 1))
                    g.wait_ge(h_done, G * (w + 1))
                    g.remote_dma_broadcast(
                        out_ap=strm[:, G * w:G * (w + 1), :],
                        in_ap=hist[:, 1 + G * w:1 + G * (w + 1), :],
                        remote_sem=rsem, local_sem=lsem,
                        rdests=[None, (0, 1)] + [None] * 6,
                    ).then_inc(psem, 1)
                    g.wait_ge(psem, w + 2)
                    g.trigger_dma(count=1)
            left = tail_delay_cycles
            while left > 0:
                cc = min(left, 1 << 20)
                g.nop(cycle_cnt=cc)
                left -= cc

        @block.tensor
        def _(te):
            te.wait_ge(dma_in, NDMA_IN * 16)
            te.wait_ge(init_ok, 3)
            aux_blk = 0

            def aux_mblock(w, m):
                nonlocal aux_blk
                p = pax[aux_blk % 2]
                if aux_blk >= 2:
                    te.wait_ge(aux_freed, aux_blk - 1)
                for k in range(KCH):
                    ins = te.matmul(p[:, 0:G], wblk(wihT, k, m),
                                    strm[:, G * w:G * (w + 1), k],
                                    start=(k == 0), stop=(k == KCH - 1))
                ins.then_inc(aux_done, 1)
                aux_blk += 1

            for w in range(min(2, NW)):
                te.wait_ge(stream_ok, w + 1)
                for m in range(MB):
                    aux_mblock(w, m)
            for t in range(nsteps):
                w, r = divmod(t, G)
                if w + 2 < NW:
                    if r == 0:
                        te.wait_ge(stream_ok, w + 3)
                    aux_mblock(w + 2, 2 * r)
                    aux_mblock(w + 2, 2 * r + 1)
                p = pch[t % 2]
                pg = pgg[t % 2]
                te.wait_ge(h_done, t)
                if t >= 2:
                    te.wait_ge(g_ready, t - 1)
                    te.wait_ge(g2_ready, t - 1)
                for j in range(24, MB):
                    for kk in range(KCH):
                        ins = te.matmul(pg[:, j - 24:j - 23], wblk(whhT, kk, j),
                                        hist[:, t, kk:kk + 1],
                                        start=(kk == 0), stop=(kk == KCH - 1))
                ins.then_inc(gpsum_g, 1)
                for j in range(24):
                    for kk in range(KCH):
                        ins = te.matmul(p[:, j:j + 1], wblk(whhT, kk, j),
                                        hist[:, t, kk:kk + 1],
                                        start=(kk == 0), stop=(kk == KCH - 1))
                ins.then_inc(gates_done, 1)

        @block.scalar
        def _(a):
            a.wait_ge(init_ok, 3)
            a.activation(dummy[:, 0:4], c_sb[:, 0:4], mybir.ActivationFunctionType.Sigmoid)
            a.activation(dummy[:, 4:8], c_sb[:, 4:8], mybir.ActivationFunctionType.Tanh)
            aux_blk = 0

            def aux_copy(w, m):
                nonlocal aux_blk
                p = pax[aux_blk % 2]
                a.wait_ge(aux_done, aux_blk + 1)
                if w >= 4:
                    a.wait_ge(h_done, G * (w - 3))
                ins = a.activation(wx[:, m, w % 4, :], p[:, 0:G],
                                   mybir.ActivationFunctionType.Identity,
                                   bias=bias[:, m:m + 1])
                ins.then_inc(aux_freed, 1)
                aux_blk += 1

            for w in range(min(2, NW)):
                for m in range(MB):
                    aux_copy(w, m)
            for t in range(nsteps):
                w, r = divmod(t, G)
                if w + 2 < NW:
                    aux_copy(w + 2, 2 * r)
                    aux_copy(w + 2, 2 * r + 1)
                a.wait_ge(g2_ready, t + 1)
                a.activation(tg_sb[:], g2_sb[:], mybir.ActivationFunctionType.Tanh).then_inc(tg_ready, 1)
                a.wait_ge(g_ready, t + 1)
                a.activation(s_sb[:], g_sb[:, 0:24], mybir.ActivationFunctionType.Sigmoid).then_inc(s_ready, 1)
                a.wait_ge(c_done, t + 1)
                a.activation(tc_sb[:], c_sb[:], mybir.ActivationFunctionType.Tanh).then_inc(tc_done, 1)

        @block.vector
        def _(v):
            v.wait_ge(init_ok, 3)
            for t in range(nsteps):
                w = t // G
                v.wait_ge(gpsum_g, t + 1)
                v.wait_ge(aux_freed, 32 * (w + 1))
                v.tensor_add(g2_sb[:], pgg[t % 2][:, 0:8], wx[:, 24:32, w % 4, t % G]).then_inc(g2_ready, 1)
                v.wait_ge(gates_done, t + 1)
                v.tensor_add(g_sb[:, 0:24], pch[t % 2][:, 0:24], wx[:, 0:24, w % 4, t % G]).then_inc(g_ready, 1)
                v.wait_ge(s_ready, t + 1)
                v.tensor_mul(t1_sb[:], s_sb[:, 8:16], c_sb[:]).then_inc(dve1, 1)
                v.wait_ge(tg_ready, t + 1)
                v.tensor_mul(t2_sb[:], s_sb[:, 0:8], tg_sb[:]).then_inc(dve1, 1)
                v.wait_ge(dve1, 2 * (t + 1))
                v.tensor_add(c_sb[:], t1_sb[:], t2_sb[:]).then_inc(c_done, 1)
                v.wait_ge(tc_done, t + 1)
                v.tensor_mul(hist[:, t + 1, :], s_sb[:, 16:24], tc_sb[:]).then_inc(h_done, 1)

    nc.compile()
    return nc


# ---------------- host-side data prep ----------------

def _reorder_gates(w):
    i, f, g, o = np.split(w, 4, axis=0)
    return np.concatenate([i, f, o, g], axis=0)


def _prep_weightT(w4h_k, kdim=1024):
    w = _reorder_gates(np.asarray(w4h_k, np.float32))
    K = w.shape[1]
    if K < kdim:
        w = np.concatenate([w, np.zeros((GD, kdim - K), np.float32)], axis=1)
    arr = w.reshape(MB, 128, KCH, 128)
    out = arr.transpose(3, 2, 0, 1).reshape(128, KCH * GD)
    return np.ascontiguousarray(out.astype(ml_dtypes.bfloat16))


def _prep_stream(x_t_d, nsteps=NSTEPS):
    x = np.asarray(x_t_d, np.float32)
    S, D = x.shape
    xp = np.zeros((nsteps, 1024), np.float32)
    xp[:S, :D] = x
    arr = xp.reshape(nsteps, 8, 128).transpose(2, 0, 1)
    return np.ascontiguousarray(arr.reshape(128, nsteps * 8).astype(ml_dtypes.bfloat16))


def _prep_bias(bih, bhh):
    b = _reorder_gates((np.asarray(bih, np.float32) + np.asarray(bhh, np.float32)).reshape(GD, 1))[:, 0]
    return np.ascontiguousarray(b.reshape(MB, 128).T.astype(np.float32))


def _core_inputs(x_dir, Wih, Whh, bih, bhh, is_l1, nsteps=NSTEPS):
    return {
        "whhT": _prep_weightT(Whh, 1024),
        "wihT": _prep_weightT(Wih, 1024),
        "strm": (np.zeros((128, nsteps * 8), ml_dtypes.bfloat16) if is_l1
                 else _prep_stream(x_dir, nsteps)),
        "bias": _prep_bias(bih, bhh),
        "role": np.array([[2 if is_l1 else 0, 0]], np.int32),
    }


def _unpack_hist(hist_out, nsteps=NSTEPS):
    a = np.asarray(hist_out).astype(np.float32).reshape(128, nsteps + 1, 8)
    return a.transpose(1, 2, 0).reshape(nsteps + 1, 1024)


def _unpack_c(c_out):
    return np.asarray(c_out, np.float32).T.reshape(1024)


_NC_CACHE = {}


def _get_nc():
    if "nc" not in _NC_CACHE:
        _NC_CACHE["nc"] = _build(NSTEPS)
    return _NC_CACHE["nc"]


def make_in_maps(last_hidden, fw0_Wih, fw0_Whh, fw0_bih, fw0_bhh,
                 fw1_Wih, fw1_Whh, fw1_bih, fw1_bhh,
                 bw0_Wih, bw0_Whh, bw0_bih, bw0_bhh,
                 bw1_Wih, bw1_Whh, bw1_bih, bw1_bhh):
    x = np.asarray(last_hidden, np.float32)[0]          # [S, IN]
    xr = x[::-1]
    m0 = _core_inputs(x, fw0_Wih, fw0_Whh, fw0_bih, fw0_bhh, is_l1=False)
    m1 = _core_inputs(x, fw1_Wih, fw1_Whh, fw1_bih, fw1_bhh, is_l1=True)
    m2 = _core_inputs(xr, bw0_Wih, bw0_Whh, bw0_bih, bw0_bhh, is_l1=False)
    m3 = _core_inputs(xr, bw1_Wih, bw1_Whh, bw1_bih, bw1_bhh, is_l1=True)
    return [m0, m1, m2, m3, m0, m1, m2, m3]


def assemble(results):
    """results: list of per-core dicts with hist_out / c_out."""
    h0f_fw = _unpack_hist(results[0]["hist_out"])[NSTEPS]
    fw_out_hist = _unpack_hist(results[1]["hist_out"])
    h1f_fw = fw_out_hist[NSTEPS]
    h0f_bw = _unpack_hist(results[2]["hist_out"])[NSTEPS]
    bw_out_hist = _unpack_hist(results[3]["hist_out"])
    h1f_bw = bw_out_hist[NSTEPS]
    c0f_fw = _unpack_c(results[0]["c_out"])
    c1f_fw = _unpack_c(results[1]["c_out"])
    c0f_bw = _unpack_c(results[2]["c_out"])
    c1f_bw = _unpack_c(results[3]["c_out"])

    cell = np.stack([np.concatenate([c0f_fw, c0f_bw]),
                     np.concatenate([c1f_fw, c1f_bw])]).astype(np.float32)
    hidden = np.stack([np.concatenate([h0f_fw, h0f_bw]),
                       np.concatenate([h1f_fw, h1f_bw])]).astype(np.float32)
    f_out = fw_out_hist[1:]                 # [S, H]
    b_out = bw_out_hist[1:][::-1]           # realign with token order
    outputs = np.concatenate([f_out, b_out], axis=1).astype(np.float32)
    return cell, hidden, outputs


def kernel(**inputs):
    nc = _get_nc()
    in_maps = make_in_maps(**{k: np.asarray(v) for k, v in inputs.items()})
    res = run_bass_kernel_spmd(nc, in_maps, list(range(8)))
    return assemble(res.results)
